# revision 5
# baseline (speedup 1.0000x reference)
"""Trainium2 Bass kernel for the BaseMemory coref scoring module.

Computes, for full inputs (M=65536 memory slots, D=768, E=20, H=64):
    score = relu(pair @ W1 + b1) @ W2 + b2, masked with ent_counter>0,
    where pair = [mem, ment, mem*ment, dist_emb, cnt_emb].

Sharding: data-parallel over the cluster dimension M across 8 NeuronCores.

Key algebraic folds (host side, O(D*H) work):
  - mem@W1_mem + (mem*ment)@W1_had = mem @ (W1_mem + diag(ment)@W1_had)
  - ment@W1_ment + b1 folded into the 10-row dist bucket table
  - bucket embedding lookups become one-hot rows contracted on the PE
  - masking folded into the PE accumulation (exact)
"""

import os
import numpy as np

M, D, E, H = 65536, 768, 20, 64
N_CORES = 8
MS = M // N_CORES          # rows per core = 8192
GROUP = 512                # rows per PE matmul group
N_GROUPS = MS // GROUP     # 16
BLK = 128                  # transpose block rows
N_BLK = MS // BLK          # 64 blocks per core
KCH = D // 128             # 6 contraction chunks
NF = 22                    # feature columns: 10 dist onehot, 10 cnt onehot, notmask, ones
BIG = float(2 ** 20)       # pre-relu kill value for masked rows

_CACHE = {}


def _build():
    """Build + compile the 8-core SPMD bass program once per process."""
    if "nc" in _CACHE:
        return _CACHE["nc"]

    import concourse.bass as bass
    import concourse.mybir as mybir
    import concourse.tile as tile
    from concourse import bacc
    from concourse.masks import make_identity

    F32 = mybir.dt.float32
    F32R = mybir.dt.float32r

    nc = bacc.Bacc("TRN2", target_bir_lowering=False, debug=False,
                   num_devices=N_CORES)

    x_d = nc.dram_tensor("x", [MS, D], F32R, kind="ExternalInput").ap()
    lms_d = nc.dram_tensor("lms", [MS], F32, kind="ExternalInput").ap()
    cnt_d = nc.dram_tensor("cnt", [MS], F32, kind="ExternalInput").ap()
    w1_d = nc.dram_tensor("w1", [D, H], F32R, kind="ExternalInput").ap()
    tcat_d = nc.dram_tensor("tcat", [NF, H], F32R, kind="ExternalInput").ap()
    scat_d = nc.dram_tensor("scat", [NF, 128], F32R, kind="ExternalInput").ap()
    w2r_d = nc.dram_tensor("w2r", [H, 128], F32R, kind="ExternalInput").ap()
    lo_d = nc.dram_tensor("lo", [128, NF], F32, kind="ExternalInput").ap()
    hi_d = nc.dram_tensor("hi", [128, NF], F32, kind="ExternalInput").ap()
    out_d = nc.dram_tensor("out", [MS], F32, kind="ExternalOutput").ap()

    # row m of the shard lives at (partition p, block t) with m = 64*p + t
    x_r = x_d.rearrange("(p t) d -> p t d", p=128)      # [128, 64, 768]
    lms_r = lms_d.rearrange("(p t) -> p t", p=128)      # [128, 64]
    cnt_r = cnt_d.rearrange("(p t) -> p t", p=128)
    w1_r = w1_d.rearrange("(k p) n -> p k n", p=128)    # [128, 6, 64]
    out_r = out_d.rearrange("(g c) -> g c", g=N_GROUPS)  # [16, 512]

    ge = mybir.AluOpType.is_ge
    le = mybir.AluOpType.is_le
    relu = mybir.ActivationFunctionType.Relu

    with tile.TileContext(nc) as tc:
        with (
            tc.tile_pool(name="consts", bufs=1) as cpool,
            tc.tile_pool(name="feat", bufs=1) as fpool,
            tc.tile_pool(name="xin", bufs=8) as px,
            tc.tile_pool(name="xt", bufs=12) as pxt,
            tc.tile_pool(name="ft", bufs=3) as pft,
            tc.tile_pool(name="ht", bufs=3) as pht,
            tc.tile_pool(name="orow", bufs=4) as pout,
            tc.tile_pool(name="pst", bufs=2, space="PSUM") as pst,
            tc.tile_pool(name="psf", bufs=2, space="PSUM") as psf,
            tc.tile_pool(name="psz", bufs=2, space="PSUM") as psz,
            tc.tile_pool(name="pss", bufs=2, space="PSUM") as pss,
        ):
            ident_t = cpool.tile([128, 128], F32, tag="ident")
            make_identity(nc, ident_t[:])
            ident_r = cpool.tile([128, 128], F32R, tag="identr")
            nc.vector.tensor_copy(ident_r[:], ident_t[:])
            ident = ident_r[:]

            w1t = cpool.tile([128, KCH, H], F32R, tag="w1t")
            nc.sync.dma_start(w1t[:], w1_r[:])
            tcat = cpool.tile([NF, H], F32R, tag="tcat")
            nc.sync.dma_start(tcat[:], tcat_d[:])
            scat = cpool.tile([NF, 128], F32R, tag="scat")
            nc.sync.dma_start(scat[:], scat_d[:])
            w2r = cpool.tile([H, 128], F32R, tag="w2r")
            nc.sync.dma_start(w2r[:], w2r_d[:])
            lo_t = cpool.tile([128, NF], F32, tag="lo")
            nc.sync.dma_start(lo_t[:], lo_d[:])
            hi_t = cpool.tile([128, NF], F32, tag="hi")
            nc.sync.dma_start(hi_t[:], hi_d[:])
            lms_t = cpool.tile([128, N_BLK], F32, tag="lms")
            nc.sync.dma_start(lms_t[:], lms_r[:])
            cnt_t = cpool.tile([128, N_BLK], F32, tag="cnt")
            nc.sync.dma_start(cnt_t[:], cnt_r[:])

            # F[p, t, i] = onehot / mask features for row m = 64p + t
            tge = fpool.tile([128, N_BLK, NF], F32, tag="tge")
            tle = fpool.tile([128, N_BLK, NF], F32, tag="tle")
            fall = fpool.tile([128, N_BLK, NF], F32R, tag="fall")
            lms_b = lms_t[:, :, None].broadcast_to([128, N_BLK, 10])
            cnt_b = cnt_t[:, :, None].broadcast_to([128, N_BLK, 12])
            nc.vector.tensor_tensor(
                tge[:, :, 0:10], lms_b,
                lo_t[:, None, 0:10].broadcast_to([128, N_BLK, 10]), ge)
            nc.vector.tensor_tensor(
                tge[:, :, 10:NF], cnt_b,
                lo_t[:, None, 10:NF].broadcast_to([128, N_BLK, 12]), ge)
            nc.vector.tensor_tensor(
                tle[:, :, 0:10], lms_b,
                hi_t[:, None, 0:10].broadcast_to([128, N_BLK, 10]), le)
            nc.vector.tensor_tensor(
                tle[:, :, 10:NF], cnt_b,
                hi_t[:, None, 10:NF].broadcast_to([128, N_BLK, 12]), le)
            nc.vector.tensor_mul(fall[:], tge[:], tle[:])

            for g in range(N_GROUPS):
                xts = []
                xtiles = []
                for j in range(4):
                    t = 4 * g + j
                    xt_in = px.tile([128, D], F32R, tag="xin")
                    nc.sync.dma_start(xt_in[:], x_r[:, t, :])
                    xtiles.append(xt_in)
                for k in range(KCH):
                    ps = pst.tile([128, GROUP], F32R, tag="pst")
                    for j in range(4):
                        nc.tensor.transpose(
                            ps[:, 128 * j:128 * (j + 1)],
                            xtiles[j][:, 128 * k:128 * (k + 1)],
                            ident)
                    xt_k = pxt.tile([128, GROUP], F32R, tag="xt")
                    if k % 2 == 0:
                        nc.vector.tensor_copy(xt_k[:], ps[:])
                    else:
                        nc.scalar.copy(xt_k[:], ps[:])
                    xts.append(xt_k)
                # feature block transposes: [128, 22] -> [22, 128]
                psft = psf.tile([NF, GROUP], F32R, tag="psf")
                for j in range(4):
                    t = 4 * g + j
                    nc.tensor.transpose(
                        psft[:, 128 * j:128 * (j + 1)],
                        fall[:, t, :], ident)
                ft = pft.tile([NF, GROUP], F32R, tag="ft")
                nc.vector.tensor_copy(ft[:], psft[:])

                zt = psz.tile([H, GROUP], F32, tag="psz")
                for k in range(KCH):
                    nc.tensor.matmul(zt[:], w1t[:, k, :], xts[k][:],
                                     start=(k == 0), stop=False)
                nc.tensor.matmul(zt[:], tcat[:], ft[:],
                                 start=False, stop=True)

                ht = pht.tile([H, GROUP], F32R, tag="ht")
                nc.scalar.activation(ht[:], zt[:], relu)

                sc = pss.tile([128, GROUP], F32, tag="pss")
                nc.tensor.matmul(sc[:], w2r[:], ht[:], start=True, stop=False)
                nc.tensor.matmul(sc[:], scat[:], ft[:], start=False, stop=True)

                orow = pout.tile([1, GROUP], F32, tag="orow")
                nc.scalar.copy(orow[:], sc[0:1, :])
                nc.sync.dma_start(out_r[g:g + 1, :], orow[:])

    nc.compile()
    _CACHE["nc"] = nc
    return nc


def _prepare_maps(ment_emb, mem_vectors, dist_table, counter_table,
                  W1, b1, W2, b2, ent_counter, last_mention_start, ment_start):
    f32 = np.float32
    ment = np.asarray(ment_emb, f32)
    mem = np.ascontiguousarray(np.asarray(mem_vectors, f32))
    W1 = np.asarray(W1, f32)
    ms = float(np.asarray(ment_start).astype(np.float64))

    W1m, W1r, W1h = W1[0:D], W1[D:2 * D], W1[2 * D:3 * D]
    W1d, W1c = W1[3 * D:3 * D + E], W1[3 * D + E:3 * D + 2 * E]

    w1eff = (W1m + ment[:, None] * W1h).astype(f32)              # [768, 64]
    bias_vec = (np.asarray(b1, f32) + ment @ W1r).astype(f32)    # [64]
    T_d = (np.asarray(dist_table, f32) @ W1d + bias_vec).astype(f32)
    T_c = (np.asarray(counter_table, f32) @ W1c).astype(f32)
    b2v = float(np.asarray(b2, f32).reshape(-1)[0])

    tcat = np.concatenate(
        [T_d, T_c, np.full((1, H), -BIG, f32), np.zeros((1, H), f32)], 0)
    scat = np.zeros((NF, 128), f32)
    scat[20, :] = -10000.0 - b2v
    scat[21, :] = b2v
    w2rep = np.repeat(np.asarray(W2, f32).reshape(H, 1), 128, axis=1)

    # bucket i covers c in [A[i], B[i]] (identity below 5, log2 above, clip 9)
    A = np.array([-1e9, 1, 2, 3, 4, 5, 8, 16, 32, 64], np.float64)
    B = np.array([0, 1, 2, 3, 4, 7, 15, 31, 63, 1e9], np.float64)
    # dist bucket in terms of lms: dist = ms - lms in [A,B] <=> lms in [ms-B, ms-A]
    lo = np.concatenate([ms - B, A, [-1e9], [-1e9]]).astype(f32)
    hi = np.concatenate([ms - A, B, [0.0], [1e9]]).astype(f32)
    lo_rep = np.ascontiguousarray(np.broadcast_to(lo, (128, NF)))
    hi_rep = np.ascontiguousarray(np.broadcast_to(hi, (128, NF)))

    lms_f = np.asarray(last_mention_start).astype(f32)
    cnt_f = np.asarray(ent_counter).astype(f32)

    in_maps = []
    for c in range(N_CORES):
        sl = slice(c * MS, (c + 1) * MS)
        in_maps.append(dict(
            x=mem[sl], lms=np.ascontiguousarray(lms_f[sl]),
            cnt=np.ascontiguousarray(cnt_f[sl]),
            w1=w1eff, tcat=tcat, scat=scat, w2r=w2rep,
            lo=lo_rep, hi=hi_rep))
    return in_maps


def _postprocess(results):
    out = np.empty(M + 1, np.float32)
    for c in range(N_CORES):
        shard = results[c]["out"]  # [8192] indexed by (g, j, p)
        out[c * MS:(c + 1) * MS] = (
            shard.reshape(N_GROUPS, 4, 128).transpose(2, 0, 1).reshape(-1))
    out[M] = 0.0
    return out


def run_spmd(in_maps, trace=False):
    from concourse.bass_utils import run_bass_kernel_spmd
    nc = _build()
    return run_bass_kernel_spmd(nc, in_maps, list(range(N_CORES)), trace=trace)


def kernel(**inputs):
    in_maps = _prepare_maps(**inputs)
    res = run_spmd(in_maps, trace=False)
    return _postprocess(res.results)


# revision 6
# speedup vs baseline: 1.0582x; 1.0582x over previous
"""Trainium2 Bass kernel for the BaseMemory coref scoring module.

Computes, for full inputs (M=65536 memory slots, D=768, E=20, H=64):
    score = relu(pair @ W1 + b1) @ W2 + b2, masked with ent_counter>0,
    where pair = [mem, ment, mem*ment, dist_emb, cnt_emb].

Sharding: data-parallel over the cluster dimension M across 8 NeuronCores.

Key algebraic folds (host side, O(D*H) work):
  - mem@W1_mem + (mem*ment)@W1_had = mem @ (W1_mem + diag(ment)@W1_had)
  - ment@W1_ment + b1 folded into the 10-row dist bucket table
  - bucket embedding lookups become one-hot rows contracted on the PE
  - masking folded into the PE accumulation (exact)
"""

import os
import numpy as np

M, D, E, H = 65536, 768, 20, 64
N_CORES = 8
MS = M // N_CORES          # rows per core = 8192
GROUP = 512                # rows per PE matmul group
N_GROUPS = MS // GROUP     # 16
BLK = 128                  # transpose block rows
N_BLK = MS // BLK          # 64 blocks per core
KCH = D // 128             # 6 contraction chunks
NF = 22                    # feature columns: 10 dist onehot, 10 cnt onehot, notmask, ones
BIG = float(2 ** 20)       # pre-relu kill value for masked rows

_CACHE = {}


def _build():
    """Build + compile the 8-core SPMD bass program once per process."""
    if "nc" in _CACHE:
        return _CACHE["nc"]

    import concourse.bass as bass
    import concourse.mybir as mybir
    import concourse.tile as tile
    from concourse import bacc
    from concourse.masks import make_identity

    F32 = mybir.dt.float32
    F32R = mybir.dt.float32r

    nc = bacc.Bacc("TRN2", target_bir_lowering=False, debug=False,
                   num_devices=N_CORES)

    x_d = nc.dram_tensor("x", [MS, D], F32R, kind="ExternalInput").ap()
    lms_d = nc.dram_tensor("lms", [MS], F32, kind="ExternalInput").ap()
    cnt_d = nc.dram_tensor("cnt", [MS], F32, kind="ExternalInput").ap()
    w1_d = nc.dram_tensor("w1", [D, H], F32R, kind="ExternalInput").ap()
    tcat_d = nc.dram_tensor("tcat", [NF, H], F32R, kind="ExternalInput").ap()
    scat_d = nc.dram_tensor("scat", [NF, 1], F32R, kind="ExternalInput").ap()
    w2r_d = nc.dram_tensor("w2r", [H, 1], F32R, kind="ExternalInput").ap()
    lo_d = nc.dram_tensor("lo", [128, NF], F32, kind="ExternalInput").ap()
    hi_d = nc.dram_tensor("hi", [128, NF], F32, kind="ExternalInput").ap()
    out_d = nc.dram_tensor("out", [MS], F32, kind="ExternalOutput").ap()

    # row m of the shard lives at (partition p, block t) with m = 64*p + t
    x_r = x_d.rearrange("(p t) d -> p t d", p=128)      # [128, 64, 768]
    lms_r = lms_d.rearrange("(p t) -> p t", p=128)      # [128, 64]
    cnt_r = cnt_d.rearrange("(p t) -> p t", p=128)
    w1_r = w1_d.rearrange("(k p) n -> p k n", p=128)    # [128, 6, 64]
    out_r = out_d.rearrange("(g c) -> g c", g=N_GROUPS)  # [16, 512]

    ge = mybir.AluOpType.is_ge
    le = mybir.AluOpType.is_le
    relu = mybir.ActivationFunctionType.Relu

    with tile.TileContext(nc) as tc:
        with (
            tc.tile_pool(name="consts", bufs=1) as cpool,
            tc.tile_pool(name="feat", bufs=1) as fpool,
            tc.tile_pool(name="xin", bufs=5) as px,
            tc.tile_pool(name="xt", bufs=12) as pxt,
            tc.tile_pool(name="ft", bufs=3) as pft,
            tc.tile_pool(name="ht", bufs=3) as pht,
            tc.tile_pool(name="orow", bufs=4) as pout,
            tc.tile_pool(name="pst", bufs=3, space="PSUM") as pst,
            tc.tile_pool(name="psf", bufs=1, space="PSUM") as psf,
            tc.tile_pool(name="psz", bufs=2, space="PSUM") as psz,
            tc.tile_pool(name="pss", bufs=2, space="PSUM") as pss,
        ):
            xin_tiles = []
            for g in range(N_GROUPS):
                xt_in = px.tile([128, 4, D], F32R, tag="xin")
                nc.sync.dma_start(xt_in[:], x_r[:, 4 * g:4 * g + 4, :])
                xin_tiles.append(xt_in)

            ident_t = cpool.tile([128, 128], F32, tag="ident")
            make_identity(nc, ident_t[:])
            ident_r = cpool.tile([128, 128], F32R, tag="identr")
            nc.vector.tensor_copy(ident_r[:], ident_t[:])
            ident = ident_r[:]

            w1t = cpool.tile([128, KCH, H], F32R, tag="w1t")
            nc.sync.dma_start(w1t[:], w1_r[:])
            tcat = cpool.tile([NF, H], F32R, tag="tcat")
            nc.sync.dma_start(tcat[:], tcat_d[:])
            scat = cpool.tile([NF, 1], F32R, tag="scat")
            nc.sync.dma_start(scat[:], scat_d[:])
            w2r = cpool.tile([H, 1], F32R, tag="w2r")
            nc.sync.dma_start(w2r[:], w2r_d[:])
            lo_t = cpool.tile([128, NF], F32, tag="lo")
            nc.sync.dma_start(lo_t[:], lo_d[:])
            hi_t = cpool.tile([128, NF], F32, tag="hi")
            nc.sync.dma_start(hi_t[:], hi_d[:])
            lms_t = cpool.tile([128, N_BLK], F32, tag="lms")
            nc.sync.dma_start(lms_t[:], lms_r[:])
            cnt_t = cpool.tile([128, N_BLK], F32, tag="cnt")
            nc.sync.dma_start(cnt_t[:], cnt_r[:])

            # F[p, t, i] = onehot / mask features for row m = 64p + t
            tge = fpool.tile([128, N_BLK, NF], F32, tag="tge")
            tle = fpool.tile([128, N_BLK, NF], F32, tag="tle")
            fall = fpool.tile([128, N_BLK, NF], F32R, tag="fall")
            lms_b = lms_t[:, :, None].broadcast_to([128, N_BLK, 10])
            cnt_b = cnt_t[:, :, None].broadcast_to([128, N_BLK, 12])
            nc.vector.tensor_tensor(
                tge[:, :, 0:10], lms_b,
                lo_t[:, None, 0:10].broadcast_to([128, N_BLK, 10]), ge)
            nc.vector.tensor_tensor(
                tge[:, :, 10:NF], cnt_b,
                lo_t[:, None, 10:NF].broadcast_to([128, N_BLK, 12]), ge)
            nc.vector.tensor_tensor(
                tle[:, :, 0:10], lms_b,
                hi_t[:, None, 0:10].broadcast_to([128, N_BLK, 10]), le)
            nc.vector.tensor_tensor(
                tle[:, :, 10:NF], cnt_b,
                hi_t[:, None, 10:NF].broadcast_to([128, N_BLK, 12]), le)
            nc.vector.tensor_mul(fall[:], tge[:], tle[:])

            for g in range(N_GROUPS):
                xts = []
                xt_in = xin_tiles[g]
                for k in range(KCH):
                    ps = pst.tile([128, GROUP], F32R, tag="pst")
                    for j in range(4):
                        nc.tensor.transpose(
                            ps[:, 128 * j:128 * (j + 1)],
                            xt_in[:, j, 128 * k:128 * (k + 1)],
                            ident)
                    xt_k = pxt.tile([128, GROUP], F32R, tag="xt")
                    if k % 2 == 0:
                        nc.vector.tensor_copy(xt_k[:], ps[:])
                    else:
                        nc.scalar.copy(xt_k[:], ps[:])
                    xts.append(xt_k)
                # feature block transposes: [128, 22] -> [22, 128]
                psft = psf.tile([NF, GROUP], F32R, tag="psf")
                for j in range(4):
                    t = 4 * g + j
                    nc.tensor.transpose(
                        psft[:, 128 * j:128 * (j + 1)],
                        fall[:, t, :], ident)
                ft = pft.tile([NF, GROUP], F32R, tag="ft")
                nc.vector.tensor_copy(ft[:], psft[:])

                zt = psz.tile([H, GROUP], F32, tag="psz")
                for k in range(KCH):
                    nc.tensor.matmul(zt[:], w1t[:, k, :], xts[k][:],
                                     start=(k == 0), stop=False)
                nc.tensor.matmul(zt[:], tcat[:], ft[:],
                                 start=False, stop=True)

                ht = pht.tile([H, GROUP], F32R, tag="ht")
                nc.scalar.activation(ht[:], zt[:], relu)

                sc = pss.tile([1, GROUP], F32, tag="pss")
                nc.tensor.matmul(sc[:], w2r[:], ht[:], start=True, stop=False)
                nc.tensor.matmul(sc[:], scat[:], ft[:], start=False, stop=True)

                orow = pout.tile([1, GROUP], F32, tag="orow")
                nc.scalar.copy(orow[:], sc[:])
                nc.sync.dma_start(out_r[g:g + 1, :], orow[:])

    nc.compile()
    _CACHE["nc"] = nc
    return nc


def _prepare_maps(ment_emb, mem_vectors, dist_table, counter_table,
                  W1, b1, W2, b2, ent_counter, last_mention_start, ment_start):
    f32 = np.float32
    ment = np.asarray(ment_emb, f32)
    mem = np.ascontiguousarray(np.asarray(mem_vectors, f32))
    W1 = np.asarray(W1, f32)
    ms = float(np.asarray(ment_start).astype(np.float64))

    W1m, W1r, W1h = W1[0:D], W1[D:2 * D], W1[2 * D:3 * D]
    W1d, W1c = W1[3 * D:3 * D + E], W1[3 * D + E:3 * D + 2 * E]

    w1eff = (W1m + ment[:, None] * W1h).astype(f32)              # [768, 64]
    bias_vec = (np.asarray(b1, f32) + ment @ W1r).astype(f32)    # [64]
    T_d = (np.asarray(dist_table, f32) @ W1d + bias_vec).astype(f32)
    T_c = (np.asarray(counter_table, f32) @ W1c).astype(f32)
    b2v = float(np.asarray(b2, f32).reshape(-1)[0])

    tcat = np.concatenate(
        [T_d, T_c, np.full((1, H), -BIG, f32), np.zeros((1, H), f32)], 0)
    scat = np.zeros((NF, 1), f32)
    scat[20, :] = -10000.0 - b2v
    scat[21, :] = b2v
    w2rep = np.asarray(W2, f32).reshape(H, 1)

    # bucket i covers c in [A[i], B[i]] (identity below 5, log2 above, clip 9)
    A = np.array([-1e9, 1, 2, 3, 4, 5, 8, 16, 32, 64], np.float64)
    B = np.array([0, 1, 2, 3, 4, 7, 15, 31, 63, 1e9], np.float64)
    # dist bucket in terms of lms: dist = ms - lms in [A,B] <=> lms in [ms-B, ms-A]
    lo = np.concatenate([ms - B, A, [-1e9], [-1e9]]).astype(f32)
    hi = np.concatenate([ms - A, B, [0.0], [1e9]]).astype(f32)
    lo_rep = np.ascontiguousarray(np.broadcast_to(lo, (128, NF)))
    hi_rep = np.ascontiguousarray(np.broadcast_to(hi, (128, NF)))

    lms_f = np.asarray(last_mention_start).astype(f32)
    cnt_f = np.asarray(ent_counter).astype(f32)

    in_maps = []
    for c in range(N_CORES):
        sl = slice(c * MS, (c + 1) * MS)
        in_maps.append(dict(
            x=mem[sl], lms=np.ascontiguousarray(lms_f[sl]),
            cnt=np.ascontiguousarray(cnt_f[sl]),
            w1=w1eff, tcat=tcat, scat=scat, w2r=w2rep,
            lo=lo_rep, hi=hi_rep))
    return in_maps


def _postprocess(results):
    out = np.empty(M + 1, np.float32)
    for c in range(N_CORES):
        shard = results[c]["out"]  # [8192] indexed by (g, j, p)
        out[c * MS:(c + 1) * MS] = (
            shard.reshape(N_GROUPS, 4, 128).transpose(2, 0, 1).reshape(-1))
    out[M] = 0.0
    return out


def run_spmd(in_maps, trace=False):
    from concourse.bass_utils import run_bass_kernel_spmd
    nc = _build()
    return run_bass_kernel_spmd(nc, in_maps, list(range(N_CORES)), trace=trace)


def kernel(**inputs):
    in_maps = _prepare_maps(**inputs)
    res = run_spmd(in_maps, trace=False)
    return _postprocess(res.results)
